# revision 1
# baseline (speedup 1.0000x reference)
# Two-layer tanh RNN (B=64, T=1024, I=256, H=512) on 8 TRN2 NeuronCores.
#
# Strategy: data-parallel over batch (8 sequences per core, weights
# replicated).  Everything stays SBUF-resident per core:
#   phase 1: xp0^T = W_ih0^T @ x^T + b0          (big GEMM, bf16)
#   phase 2: layer-0 scan over T in transposed state layout
#            h^T[128, t, 4*8]  (partition = H%128, free = (H//128, batch))
#            per step: PSUM seeded with xp^T via an identity matmul, then
#            16 bf16 [128x128] W_hh matmuls accumulate, one Tanh on ScalarE
#   phase 3: xp1^T = W_ih1^T @ h0^T + b1         (big GEMM from SBUF ring)
#   phase 4: layer-1 scan (same structure)
#   phase 5: DMA out h1 ring (= out1^T) and final h states
#
# Matmuls run in bf16 (stationary + moving), fp32 PSUM accumulation.
# Validated offline: rel err vs fp64 reference ~5e-3, flat over T (the
# tanh recurrence is contractive, bf16 error does not compound).

import numpy as np
import ml_dtypes

B, T_FULL, I, H = 64, 1024, 256, 512
N_CORES = 8
BS = B // N_CORES          # batch per core = 8
G = H // 128               # H chunks = 4
GW = G * BS                # packed free width = 32
KI = I // 128              # input K chunks = 2

BF16 = ml_dtypes.bfloat16

_nc_cache = {}


def _build_nc(T):
    import concourse.mybir as mybir
    import concourse.tile as tile
    from concourse import bacc

    dt = mybir.dt
    AF = mybir.ActivationFunctionType

    NT = (T * BS) // 512   # row tiles of 512 in the big GEMMs
    TPT = 512 // BS        # timesteps per row tile = 64

    nc = bacc.Bacc(None, target_bir_lowering=False)

    xT_d = nc.dram_tensor("xT", [KI, 128, T, BS], dt.bfloat16, kind="ExternalInput")
    wih0_d = nc.dram_tensor("Wih0", [128, KI, G, 128], dt.bfloat16, kind="ExternalInput")
    whh0_d = nc.dram_tensor("Whh0", [128, G, G, 128], dt.bfloat16, kind="ExternalInput")
    wih1_d = nc.dram_tensor("Wih1", [128, G, G, 128], dt.bfloat16, kind="ExternalInput")
    whh1_d = nc.dram_tensor("Whh1", [128, G, G, 128], dt.bfloat16, kind="ExternalInput")
    b0_d = nc.dram_tensor("b0v", [128, G], dt.float32, kind="ExternalInput")
    b1_d = nc.dram_tensor("b1v", [128, G], dt.float32, kind="ExternalInput")
    ident_d = nc.dram_tensor("ident", [128, 128], dt.bfloat16, kind="ExternalInput")

    out1T_d = nc.dram_tensor("out1T", [128, T, GW], dt.bfloat16, kind="ExternalOutput")
    hn_d = nc.dram_tensor("hn", [128, 2, GW], dt.bfloat16, kind="ExternalOutput")

    with tile.TileContext(nc) as tc:
        with (
            tc.tile_pool(name="consts", bufs=1) as consts,
            tc.tile_pool(name="big", bufs=1) as big,
            tc.tile_pool(name="psg", bufs=4, space="PSUM") as psg,
            tc.tile_pool(name="pss", bufs=4, space="PSUM") as pss,
        ):
            wih0 = consts.tile([128, KI, G, 128], dt.bfloat16)
            whh0 = consts.tile([128, G, G, 128], dt.bfloat16)
            wih1 = consts.tile([128, G, G, 128], dt.bfloat16)
            whh1 = consts.tile([128, G, G, 128], dt.bfloat16)
            b0v = consts.tile([128, G], dt.float32)
            b1v = consts.tile([128, G], dt.float32)
            ident = consts.tile([128, 128], dt.bfloat16)
            hzero = consts.tile([128, GW], dt.bfloat16)
            h0last = consts.tile([128, GW], dt.bfloat16)

            xt = big.tile([128, KI, T, BS], dt.bfloat16)
            xp = big.tile([128, T, GW], dt.bfloat16)
            hring = big.tile([128, T, GW], dt.bfloat16)

            nc.sync.dma_start(wih0[:], wih0_d[:])
            nc.sync.dma_start(whh0[:], whh0_d[:])
            nc.sync.dma_start(wih1[:], wih1_d[:])
            nc.sync.dma_start(whh1[:], whh1_d[:])
            nc.sync.dma_start(b0v[:], b0_d[:])
            nc.sync.dma_start(b1v[:], b1_d[:])
            nc.sync.dma_start(ident[:], ident_d[:])
            for k in range(KI):
                nc.sync.dma_start(xt[:, k, :, :], xT_d[k, :, :, :])
            nc.vector.memset(hzero[:], 0.0)

            def big_gemm(w_sb, nk, src_rhs, bias_sb):
                # xp[:, trange, c*BS:(c+1)*BS] = sum_k w[:,k,c,:].T @ rhs_k + bias_c
                for c in range(G):
                    for n in range(NT):
                        ps = psg.tile([128, TPT, BS], dt.float32, tag="psg")
                        tr = slice(n * TPT, (n + 1) * TPT)
                        for k in range(nk):
                            nc.tensor.matmul(
                                ps[:],
                                w_sb[:, k, c, :],
                                src_rhs(k, tr),
                                start=(k == 0),
                                stop=(k == nk - 1),
                            )
                        nc.scalar.activation(
                            xp[:, tr, c * BS:(c + 1) * BS],
                            ps[:],
                            AF.Identity,
                            bias=bias_sb[:, c:c + 1],
                        )

            def scan(w_sb):
                prev = hzero
                for t in range(T):
                    ps = pss.tile([128, GW], dt.float32, tag="pss")
                    nc.tensor.matmul(ps[:], ident[:], xp[:, t, :],
                                     start=True, stop=False)
                    for m in range(G):
                        for k in range(G):
                            nc.tensor.matmul(
                                ps[:, m * BS:(m + 1) * BS],
                                w_sb[:, k, m, :],
                                prev[:, k * BS:(k + 1) * BS],
                                start=False,
                                stop=(m == G - 1 and k == G - 1),
                                skip_group_check=True,
                            )
                    nc.scalar.activation(hring[:, t, :], ps[:], AF.Tanh)
                    prev = hring[:, t, :]

            # phase 1 + 2: layer 0
            big_gemm(wih0, KI, lambda k, tr: xt[:, k, tr, :], b0v)
            scan(whh0)
            nc.vector.tensor_copy(h0last[:], hring[:, T - 1, :])

            # phase 3 + 4: layer 1
            big_gemm(wih1, G, lambda k, tr: hring[:, tr, k * BS:(k + 1) * BS], b1v)
            scan(whh1)

            # phase 5: outputs
            nc.sync.dma_start(hn_d[:, 0, :], h0last[:])
            nc.sync.dma_start(hn_d[:, 1, :], hring[:, T - 1, :])
            nchunk = 8 if T % 8 == 0 else 1
            for ch in range(nchunk):
                ts = slice(ch * T // nchunk, (ch + 1) * T // nchunk)
                nc.sync.dma_start(out1T_d[:, ts, :], hring[:, ts, :])

    nc.compile()
    return nc


def _get_nc(T):
    if T not in _nc_cache:
        _nc_cache[T] = _build_nc(T)
    return _nc_cache[T]


def _prep_shared(W_ih0, W_hh0, b0, W_ih1, W_hh1, b1):
    def wtiles(W, nk):
        return np.ascontiguousarray(
            W.reshape(nk, 128, G, 128).transpose(1, 0, 2, 3)
        ).astype(BF16)

    return {
        "Wih0": wtiles(np.asarray(W_ih0, np.float32), KI),
        "Whh0": wtiles(np.asarray(W_hh0, np.float32), G),
        "Wih1": wtiles(np.asarray(W_ih1, np.float32), G),
        "Whh1": wtiles(np.asarray(W_hh1, np.float32), G),
        "b0v": np.ascontiguousarray(np.asarray(b0, np.float32).reshape(G, 128).T),
        "b1v": np.ascontiguousarray(np.asarray(b1, np.float32).reshape(G, 128).T),
        "ident": np.eye(128, dtype=BF16),
    }


def _run(x, W_ih0, W_hh0, b0, W_ih1, W_hh1, b1, T=T_FULL, trace=False):
    from concourse.bass_utils import run_bass_kernel_spmd

    nc = _get_nc(T)
    x = np.asarray(x, np.float32)
    shared = _prep_shared(W_ih0, W_hh0, b0, W_ih1, W_hh1, b1)
    in_maps = []
    for c in range(N_CORES):
        xs = x[c * BS:(c + 1) * BS, :T]                        # [BS, T, I]
        xT = np.ascontiguousarray(xs.transpose(2, 1, 0)).astype(BF16)
        in_maps.append({"xT": xT.reshape(KI, 128, T, BS), **shared})

    res = run_bass_kernel_spmd(nc, in_maps, core_ids=list(range(N_CORES)),
                               trace=trace)

    outs, hns = [], []
    for c in range(N_CORES):
        o = res.results[c]["out1T"].astype(np.float32)         # [128, T, GW]
        o = o.reshape(128, T, G, BS).transpose(3, 1, 2, 0).reshape(BS, T, H)
        outs.append(o)
        hn = res.results[c]["hn"].astype(np.float32)           # [128, 2, GW]
        hn = hn.reshape(128, 2, G, BS).transpose(1, 3, 2, 0).reshape(2, BS, H)
        hns.append(hn)
    out1 = np.concatenate(outs, axis=0)
    h_n = np.concatenate(hns, axis=1)
    return (out1, h_n), res


def kernel(x, W_ih0, W_hh0, b0, W_ih1, W_hh1, b1):
    (out1, h_n), _ = _run(x, W_ih0, W_hh0, b0, W_ih1, W_hh1, b1)
    return out1, h_n


# revision 2
# speedup vs baseline: 1662.1073x; 1662.1073x over previous
# Two-layer tanh RNN (B=64, T=1024, I=256, H=512) on 8 TRN2 NeuronCores.
#
# Strategy: data-parallel over batch (8 sequences per core, weights
# replicated).  Everything stays SBUF-resident per core:
#   phase 1: xp0^T = W_ih0^T @ x^T + b0          (big GEMM, bf16)
#   phase 2: layer-0 scan over T in transposed state layout
#            h^T[128, t, 4*8]  (partition = H%128, free = (H//128, batch))
#            per step: PSUM seeded with xp^T via an identity matmul, then
#            16 bf16 [128x128] W_hh matmuls accumulate, tanh on ScalarE
#   phase 3: xp1^T = W_ih1^T @ h0^T + b1         (big GEMM from SBUF ring)
#   phase 4: layer-1 scan (same structure)
#   phase 5: DMA out h1 ring (= out1^T) and final h states
#
# Matmuls run in bf16 (stationary + moving), fp32 PSUM accumulation.
# Validated: rel err vs fp32 reference ~5.9e-3, flat over T (the tanh
# recurrence is contractive, bf16 error does not compound).

import numpy as np
import ml_dtypes

B, T_FULL, I, H = 64, 1024, 256, 512
N_CORES = 8
BS = B // N_CORES          # batch per core = 8
G = H // 128               # H chunks = 4
GW = G * BS                # packed free width = 32
HB = GW // 2               # half width = 16
KI = I // 128              # input K chunks = 2

BF16 = ml_dtypes.bfloat16

_nc_cache = {}
_fn_cache = {}


def _build_nc(T, reps_loop=False):
    import concourse.mybir as mybir
    import concourse.tile as tile
    from concourse import bacc

    dt = mybir.dt
    AF = mybir.ActivationFunctionType

    NT = (T * BS) // 512   # row tiles of 512 in the big GEMMs
    TPT = 512 // BS        # timesteps per row tile = 64

    nc = bacc.Bacc(None, target_bir_lowering=False)

    xT_d = nc.dram_tensor("xT", [KI, 128, T, BS], dt.bfloat16, kind="ExternalInput")
    wih0_d = nc.dram_tensor("Wih0", [128, KI, G, 128], dt.bfloat16, kind="ExternalInput")
    whh0_d = nc.dram_tensor("Whh0", [128, G, G, 128], dt.bfloat16, kind="ExternalInput")
    wih1_d = nc.dram_tensor("Wih1", [128, G, G, 128], dt.bfloat16, kind="ExternalInput")
    whh1_d = nc.dram_tensor("Whh1", [128, G, G, 128], dt.bfloat16, kind="ExternalInput")
    b0_d = nc.dram_tensor("b0v", [128, G], dt.float32, kind="ExternalInput")
    b1_d = nc.dram_tensor("b1v", [128, G], dt.float32, kind="ExternalInput")
    ident_d = nc.dram_tensor("ident", [128, 128], dt.bfloat16, kind="ExternalInput")
    if reps_loop:
        reps_d = nc.dram_tensor("reps", [1, 1], dt.int32, kind="ExternalInput")

    out1T_d = nc.dram_tensor("out1T", [128, T, GW], dt.bfloat16, kind="ExternalOutput")
    hn_d = nc.dram_tensor("hn", [128, 2, GW], dt.bfloat16, kind="ExternalOutput")

    with tile.TileContext(nc) as tc:
        with (
            tc.tile_pool(name="consts", bufs=1) as consts,
            tc.tile_pool(name="big", bufs=1) as big,
            tc.tile_pool(name="psg", bufs=4, space="PSUM") as psg,
            tc.tile_pool(name="pss", bufs=4, space="PSUM") as pss,
        ):
            wih0 = consts.tile([128, KI, G, 128], dt.bfloat16)
            whh0 = consts.tile([128, G, G, 128], dt.bfloat16)
            wih1 = consts.tile([128, G, G, 128], dt.bfloat16)
            whh1 = consts.tile([128, G, G, 128], dt.bfloat16)
            b0v = consts.tile([128, G], dt.float32)
            b1v = consts.tile([128, G], dt.float32)
            ident = consts.tile([128, 128], dt.bfloat16)
            hzero = consts.tile([128, GW], dt.bfloat16)
            h0last = consts.tile([128, GW], dt.bfloat16)

            xt = big.tile([128, KI, T, BS], dt.bfloat16)
            xp = big.tile([128, T, GW], dt.bfloat16)
            hring = big.tile([128, T, GW], dt.bfloat16)

            nc.sync.dma_start(wih0[:], wih0_d[:])
            nc.sync.dma_start(whh0[:], whh0_d[:])
            nc.sync.dma_start(wih1[:], wih1_d[:])
            nc.sync.dma_start(whh1[:], whh1_d[:])
            nc.sync.dma_start(b0v[:], b0_d[:])
            nc.sync.dma_start(b1v[:], b1_d[:])
            nc.sync.dma_start(ident[:], ident_d[:])
            for k in range(KI):
                nc.sync.dma_start(xt[:, k, :, :], xT_d[k, :, :, :])
            nc.vector.memset(hzero[:], 0.0)

            def big_gemm(w_sb, nk, src_rhs, bias_sb):
                # xp[:, tr, c*BS:(c+1)*BS] = sum_k w[:,k,c,:].T @ rhs_k + bias_c
                for c in range(G):
                    for n in range(NT):
                        ps = psg.tile([128, TPT, BS], dt.float32, tag="psg",
                                      name="psgt")
                        tr = slice(n * TPT, (n + 1) * TPT)
                        for k in range(nk):
                            nc.tensor.matmul(
                                ps[:],
                                w_sb[:, k, c, :],
                                src_rhs(k, tr),
                                start=(k == 0),
                                stop=(k == nk - 1),
                            )
                        dst = xp[:, tr, c * BS:(c + 1) * BS]
                        if n % 2 == 0:
                            nc.scalar.activation(dst, ps[:], AF.Identity,
                                                 bias=bias_sb[:, c:c + 1])
                        else:
                            nc.vector.tensor_scalar_add(dst, ps[:],
                                                        bias_sb[:, c:c + 1])

            def scan(w_sb):
                prev = hzero
                for t in range(T):
                    ps = pss.tile([128, GW], dt.float32, tag="pss", name="psst")
                    nc.tensor.matmul(ps[:], ident[:], xp[:, t, :],
                                     start=True, stop=False)
                    for m in range(G):
                        for k in range(G):
                            nc.tensor.matmul(
                                ps[:, m * BS:(m + 1) * BS],
                                w_sb[:, k, m, :],
                                prev[:, k * BS:(k + 1) * BS],
                                start=False,
                                stop=(m == G - 1 and k == G - 1),
                                skip_group_check=True,
                            )
                    nc.scalar.activation(hring[:, t, :], ps[:], AF.Tanh)
                    prev = hring[:, t, :]

            def body():
                big_gemm(wih0, KI, lambda k, tr: xt[:, k, tr, :], b0v)
                scan(whh0)
                nc.vector.tensor_copy(h0last[:], hring[:, T - 1, :])
                big_gemm(wih1, G,
                         lambda k, tr: hring[:, tr, k * BS:(k + 1) * BS], b1v)
                scan(whh1)
                nc.sync.dma_start(hn_d[:, 0, :], h0last[:])
                nc.sync.dma_start(hn_d[:, 1, :], hring[:, T - 1, :])
                nchunk = 8 if T % 8 == 0 else 1
                for ch in range(nchunk):
                    ts = slice(ch * T // nchunk, (ch + 1) * T // nchunk)
                    nc.sync.dma_start(out1T_d[:, ts, :], hring[:, ts, :])

            if reps_loop:
                import concourse.mybir as _mybir
                tmp = nc.alloc_registers("reps_regs", _mybir.ALL_ENGINES)
                nc.regs_load(tmp, reps_d[0:1, 0:1])
                rv = nc.snap(tmp, donate=True, min_val=0, max_val=1 << 20)
                with tc.For_i(0, rv):
                    body()
            else:
                body()

    nc.compile()
    return nc


def _get_nc(T, reps_loop=False):
    key = (T, reps_loop)
    if key not in _nc_cache:
        _nc_cache[key] = _build_nc(T, reps_loop)
    return _nc_cache[key]


def _make_fn(nc):
    """Persistent sharded jit executable for an nc (avoids per-call NEFF
    reload in run_bass_kernel_spmd)."""
    import jax
    from jax.sharding import Mesh, PartitionSpec
    from jax.experimental.shard_map import shard_map
    import concourse.mybir as mybir
    from concourse.bass2jax import (_bass_exec_p, install_neuronx_cc_hook,
                                    partition_id_tensor)

    install_neuronx_cc_hook()
    in_names, out_names, out_avals = [], [], []
    pname = nc.partition_id_tensor.name if nc.partition_id_tensor else None
    for alloc in nc.m.functions[0].allocations:
        if not isinstance(alloc, mybir.MemoryLocationSet):
            continue
        name = alloc.memorylocations[0].name
        if alloc.kind == "ExternalInput":
            if name != pname:
                in_names.append(name)
        elif alloc.kind == "ExternalOutput":
            out_names.append(name)
            shape = tuple(alloc.tensor_shape)
            dtp = mybir.dt.np(alloc.dtype)
            out_avals.append(jax.core.ShapedArray(shape, dtp))
    n_params = len(in_names)
    all_names = in_names + out_names
    if pname is not None:
        all_names = all_names + [pname]
    donate = tuple(range(n_params, n_params + len(out_names)))

    def _body(*args):
        operands = list(args)
        if pname is not None:
            operands.append(partition_id_tensor())
        outs = _bass_exec_p.bind(
            *operands, out_avals=tuple(out_avals), in_names=tuple(all_names),
            out_names=tuple(out_names), lowering_input_output_aliases=(),
            sim_require_finite=True, sim_require_nnan=True, nc=nc)
        return tuple(outs)

    devices = jax.devices()[:N_CORES]
    mesh = Mesh(np.asarray(devices), ("core",))
    specs = (PartitionSpec("core"),)
    fn = jax.jit(
        shard_map(_body, mesh=mesh,
                  in_specs=specs * (n_params + len(out_names)),
                  out_specs=specs * len(out_names), check_rep=False),
        donate_argnums=donate, keep_unused=True)
    return fn, in_names, out_names, out_avals


def _get_fn(T, reps_loop=False):
    key = (T, reps_loop)
    if key not in _fn_cache:
        _fn_cache[key] = _make_fn(_get_nc(T, reps_loop))
    return _fn_cache[key]


def _prep_shared(W_ih0, W_hh0, b0, W_ih1, W_hh1, b1):
    def wtiles(W, nk):
        return np.ascontiguousarray(
            W.reshape(nk, 128, G, 128).transpose(1, 0, 2, 3)
        ).astype(BF16)

    return {
        "Wih0": wtiles(np.asarray(W_ih0, np.float32), KI),
        "Whh0": wtiles(np.asarray(W_hh0, np.float32), G),
        "Wih1": wtiles(np.asarray(W_ih1, np.float32), G),
        "Whh1": wtiles(np.asarray(W_hh1, np.float32), G),
        "b0v": np.ascontiguousarray(np.asarray(b0, np.float32).reshape(G, 128).T),
        "b1v": np.ascontiguousarray(np.asarray(b1, np.float32).reshape(G, 128).T),
        "ident": np.eye(128, dtype=BF16),
    }


def _exec(T, in_maps, reps=None):
    """Run via the cached jit executable. in_maps: list of 8 dicts."""
    reps_loop = reps is not None
    fn, in_names, out_names, out_avals = _get_fn(T, reps_loop)
    if reps_loop:
        for m in in_maps:
            m["reps"] = np.full((1, 1), reps, np.int32)
    concat_in = [np.concatenate([m[n] for m in in_maps], axis=0)
                 for n in in_names]
    zouts = [np.zeros((N_CORES * a.shape[0], *a.shape[1:]), a.dtype)
             for a in out_avals]
    outs = fn(*concat_in, *zouts)
    outs = [np.asarray(o) for o in outs]
    results = []
    for c in range(N_CORES):
        results.append({
            name: outs[i].reshape(N_CORES, *out_avals[i].shape)[c]
            for i, name in enumerate(out_names)})
    return results


def _make_in_maps(x, shared, T):
    in_maps = []
    for c in range(N_CORES):
        xs = x[c * BS:(c + 1) * BS, :T]                        # [BS, T, I]
        xT = np.ascontiguousarray(xs.transpose(2, 1, 0)).astype(BF16)
        in_maps.append({"xT": xT.reshape(KI, 128, T, BS), **shared})
    return in_maps


def _gather(results, T):
    outs, hns = [], []
    for c in range(N_CORES):
        o = results[c]["out1T"].astype(np.float32)             # [128, T, GW]
        o = o.reshape(128, T, G, BS).transpose(3, 1, 2, 0).reshape(BS, T, H)
        outs.append(o)
        hn = results[c]["hn"].astype(np.float32)               # [128, 2, GW]
        hn = hn.reshape(128, 2, G, BS).transpose(1, 3, 2, 0).reshape(2, BS, H)
        hns.append(hn)
    return np.concatenate(outs, axis=0), np.concatenate(hns, axis=1)


def _run(x, W_ih0, W_hh0, b0, W_ih1, W_hh1, b1, T=T_FULL, reps=None):
    x = np.asarray(x, np.float32)
    shared = _prep_shared(W_ih0, W_hh0, b0, W_ih1, W_hh1, b1)
    in_maps = _make_in_maps(x, shared, T)
    results = _exec(T, in_maps, reps=reps)
    return _gather(results, T)


def kernel(x, W_ih0, W_hh0, b0, W_ih1, W_hh1, b1):
    out1, h_n = _run(x, W_ih0, W_hh0, b0, W_ih1, W_hh1, b1)
    return out1, h_n
